# revision 4
# baseline (speedup 1.0000x reference)
"""Trainium2 kernel for nn_DoubleAffineNet.

Math: the module's output is phi + psi - I where phi, psi are 3x3 affine
matrices built from pooled image statistics. phi needs mean(x), mean(y).
psi needs mean(x) and mean(y_comp), where y_comp is y bilinearly warped by
the near-identity affine map phi^{-1}.

Key identity: only the MEAN of y_comp is needed. Writing the warp-mean as
sum_{p,q} Y[p,q] * G[p,q] (G = bilinear splat weights of the affinely
mapped output lattice), a partition-of-unity argument shows that for
sub-pixel displacement fields (|u|,|v| < 0.5, which holds for this
problem's near-identity maps; asserted at runtime on the host), G is the
constant kappa = (1-a')(1-d') + b*c everywhere except the four border
rows/cols. Hence

    sum(y_comp) = kappa * sum(y) + sum_border Y*(G_true - kappa)

The device kernel therefore only computes the memory-bound statistics:
per-sample sum(x), sum(y), and the four border strips of y. The remaining
O(B*(3x3 + 4*1024)) algebra runs on the host (f32 where the reference is
f32, f64 for the border correction).

Sharding: pure data parallel, one sample per NeuronCore (B=8, 8 cores).

Device program (raw bacc, no Block, no end-of-block barrier): the NEFF
epilogue that walrus appends clears all 253 semaphores, split ~51 per
sequencer, serially (~2.2-5.9us per engine). Without a user barrier each
engine falls into its share of that sweep right after its OWN last
instruction, so the sweep overlaps the DMA stream instead of serializing
after it. Late-landing chunks are reduced by the engines whose sweep
share is fastest (Sync 2.2us > can't reduce; Vector 3.5us; Scalar 4.6us;
Tensor 5.9us stays idle), and the input stream is split across both
HWDGE rings (Sync + Scalar) so ring boundaries interleave.

  - inputs as 11 chunks; x and early y reduced by Scalar (ACT accum),
    late y by Vector (tensor_reduce); GpSimd copies the border-column
    strips; row strips go DRAM->DRAM directly, issued early
  - everything lands in one packed [128, 28] "smalls" tile
  - host does the final ~KB of reduction/algebra in float64
"""

import numpy as np

H = 1024
W = 1024
OUT_LEN = 5632
SMALLS_COLS = 28

_CACHE = {}


def _build_program():
    import contextlib

    import concourse.bacc as bacc
    from concourse import mybir

    f32 = mybir.dt.float32
    Copy = mybir.ActivationFunctionType.Copy
    nc = bacc.Bacc("TRN2", target_bir_lowering=False, debug=False, num_devices=8)

    xd = nc.dram_tensor("x", [H, W], f32, kind="ExternalInput").ap()
    yd = nc.dram_tensor("y", [H, W], f32, kind="ExternalInput").ap()
    outd = nc.dram_tensor("out", [OUT_LEN], f32, kind="ExternalOutput").ap()

    # chunk table: (name, src_ap builder, sbuf free-size, ring, reducer)
    # c0..c3: x rows k*256..k*256+255 as [128, 2, 1024]
    # c4, c5: y rows 0..255 / 256..511 as [128, 2, 1024]
    # c6..c8: y rows 512..639 / 640..767 / 768..895 as [128, 1024]
    # c9:     y rows 896..1023, cols 0..767  as [128, 768]
    # c10:    y rows 896..1023, cols 768..1023 as [128, 256]
    def src(k):
        if k < 4:
            return xd[k * 256:(k + 1) * 256, :].rearrange("(a p) q -> p a q", a=2)
        if k < 6:
            c = k - 4
            return yd[c * 256:(c + 1) * 256, :].rearrange("(a p) q -> p a q", a=2)
        if k < 9:
            r = 512 + (k - 6) * 128
            return yd[r:r + 128, :]
        if k == 9:
            return yd[896:1024, 0:768]
        return yd[896:1024, 768:1024]

    free = [2048, 2048, 2048, 2048, 2048, 2048, 1024, 1024, 1024, 768, 256]
    # issue ring per chunk: pair up Sync/Scalar so the SDMA engines
    # round-robin between the two HWDGE rings and chunk boundaries on one
    # ring are covered by the other
    sync_chunks = [0, 2, 4, 6, 8, 9, 10]
    scalar_chunks = [1, 3, 5, 7]
    scalar_red = [0, 2, 4, 6]           # ACT accum, all land by ~29us
    vector_red = [1, 3, 5, 7, 8, 9, 10]  # DVE reduce, takes the tail

    with contextlib.ExitStack() as ctx:
        bufs = [
            ctx.enter_context(nc.sbuf_tensor(f"chunk{k}", [128, free[k]], f32))
            for k in range(11)
        ]
        # smalls cols: 0..3 x partials, 4..10 y partials (c4..c10),
        # 11 unused, 12..19 col0 strip by 128-row block, 20..27 col1023
        smalls = ctx.enter_context(nc.sbuf_tensor("smalls", [128, SMALLS_COLS], f32))
        scratch = ctx.enter_context(nc.sbuf_tensor("scratch", [128, 2048], f32))
        sem_in = [ctx.enter_context(nc.semaphore(f"in{k}")) for k in range(11)]
        done = ctx.enter_context(nc.semaphore("done"))
        dma_out = ctx.enter_context(nc.semaphore("dma_out"))

        def dst(k):
            if k < 6:
                return bufs[k].ap().rearrange("p (a q) -> p a q", a=2)
            return bufs[k][:]

        # ---- Sync: issue its ring's input chunks, then the output ----
        for k in sync_chunks:
            nc.sync.dma_start(out=dst(k), in_=src(k)).then_inc(sem_in[k], 16)

        # ---- Scalar ring: its input chunks + the row strips (DRAM->DRAM),
        # all issued before Scalar's first reduce wait ----
        for k in scalar_chunks:
            nc.scalar.dma_start(out=dst(k), in_=src(k)).then_inc(sem_in[k], 16)
        nc.scalar.dma_start(
            out=outd[3584:4608].rearrange("(p q) -> p q", p=1),
            in_=yd[0:1, :],
        ).then_inc(dma_out, 16)
        nc.scalar.dma_start(
            out=outd[4608:5632].rearrange("(p q) -> p q", p=1),
            in_=yd[1023:1024, :],
        ).then_inc(dma_out, 16)

        # ---- Scalar reduces (ACT accumulate) ----
        for j, k in enumerate(scalar_red):
            nc.scalar.wait_ge(sem_in[k], 16)
            inst = nc.scalar.activation(
                scratch[:, 0:free[k]], bufs[k][:], Copy,
                accum_out=smalls[:, k:k + 1] if k < 4 else smalls[:, 4 + (k - 4):5 + (k - 4)],
            )
            if j == len(scalar_red) - 1:
                inst.then_inc(done, 1)

        # ---- Vector reduces ----
        for j, k in enumerate(vector_red):
            nc.vector.wait_ge(sem_in[k], 16)
            col = k if k < 4 else 4 + (k - 4)
            inst = nc.vector.tensor_reduce(
                out=smalls[:, col:col + 1],
                in_=bufs[k][:],
                axis=mybir.AxisListType.X,
                op=mybir.AluOpType.add,
            )
            if j == len(vector_red) - 1:
                inst.then_inc(done, 1)

        # ---- GpSimd: border-column strips ----
        # smalls[:, 12+blk] = y[blk*128+p, 0]; smalls[:, 20+blk] = y[.., 1023]
        # c4 covers blocks 0-1, c5 blocks 2-3 (two cols each); c6/c7/c8
        # blocks 4/5/6; block 7's col0 lives in c9, col1023 in c10.
        last_inst = None
        for k in (4, 5):
            nc.gpsimd.wait_ge(sem_in[k], 16)
            t3 = bufs[k].ap().rearrange("p (a q) -> p a q", a=2)
            j = 12 + 2 * (k - 4)
            nc.gpsimd.tensor_copy(smalls[:, j:j + 2], t3[:, :, 0])
            last_inst = nc.gpsimd.tensor_copy(smalls[:, j + 8:j + 10], t3[:, :, 1023])
        for k in (6, 7, 8):
            nc.gpsimd.wait_ge(sem_in[k], 16)
            j = 16 + (k - 6)
            nc.gpsimd.tensor_copy(smalls[:, j:j + 1], bufs[k][:, 0:1])
            last_inst = nc.gpsimd.tensor_copy(
                smalls[:, j + 8:j + 9], bufs[k][:, 1023:1024]
            )
        nc.gpsimd.wait_ge(sem_in[9], 16)
        nc.gpsimd.tensor_copy(smalls[:, 19:20], bufs[9][:, 0:1])
        nc.gpsimd.wait_ge(sem_in[10], 16)
        last_inst = nc.gpsimd.tensor_copy(smalls[:, 27:28], bufs[10][:, 255:256])
        last_inst.then_inc(done, 1)

        # ---- Sync: the one smalls output DMA, gated on all reducers ----
        nc.sync.wait_ge(done, 3)
        nc.sync.dma_start(
            out=outd[0:3584].rearrange("(p c) -> p c", c=SMALLS_COLS),
            in_=smalls[:],
        ).then_inc(dma_out, 16)
        # prove all three output DMAs landed before Sync retires
        nc.sync.wait_ge(dma_out, 48)

    nc.compile()
    return nc


def _get_program():
    if "nc" not in _CACHE:
        _CACHE["nc"] = _build_program()
    return _CACHE["nc"]


def _tent(z):
    return np.maximum(0.0, 1.0 - np.abs(z))


def _warp_mean_exact(y_img, A):
    """Fallback: honest bilinear warp-mean in numpy (used only if the
    sub-pixel displacement assumption fails, which it does not for this
    problem's inputs)."""
    A64 = A.astype(np.float64)
    i = np.arange(H, dtype=np.float64)[:, None]
    j = np.arange(W, dtype=np.float64)[None, :]
    px = A64[0, 0] * i + A64[0, 1] * j + 1023.0 * A64[0, 2]
    py = A64[1, 0] * i + A64[1, 1] * j + 1023.0 * A64[1, 2]
    x0 = np.floor(px).astype(np.int64)
    y0 = np.floor(py).astype(np.int64)
    wx = px - x0
    wy = py - y0
    im = y_img.astype(np.float64)
    acc = np.zeros((H, W))
    for xi, yi, w in (
        (x0, y0, (1 - wx) * (1 - wy)),
        (x0, y0 + 1, (1 - wx) * wy),
        (x0 + 1, y0, wx * (1 - wy)),
        (x0 + 1, y0 + 1, wx * wy),
    ):
        valid = (xi >= 0) & (xi < H) & (yi >= 0) & (yi < W)
        acc += im[np.clip(xi, 0, H - 1), np.clip(yi, 0, W - 1)] * w * valid
    return acc.mean()


def _warp_sum(sum_y, row0, row1, c0, c1, A):
    """sum(y_comp) from sum(y) + border strips, given phi_inv = A (f32).

    Requires the sub-pixel displacement assumption |u|,|v| < 0.5 (checked
    at the field corners; the fields are affine so corners bound the
    interior). The caller falls back to _warp_mean_exact otherwise.
    """
    A64 = A.astype(np.float64)
    ap, bb = A64[0, 0] - 1.0, A64[0, 1]
    cc, dp = A64[1, 0], A64[1, 1] - 1.0
    e1, e2 = 1023.0 * A64[0, 2], 1023.0 * A64[1, 2]

    mu = max(abs(ap * i + bb * j + e1) for i in (0.0, 1023.0) for j in (0.0, 1023.0))
    mv = max(abs(cc * i + dp * j + e2) for i in (0.0, 1023.0) for j in (0.0, 1023.0))
    assert mu < 0.5 and mv < 0.5, (mu, mv)

    kappa = (1.0 - ap) * (1.0 - dp) + bb * cc

    def g_true(p, q):
        g = np.zeros(np.broadcast(p, q).shape)
        for di in (-1, 0, 1):
            for dj in (-1, 0, 1):
                i_, j_ = p - di, q - dj
                valid = (i_ >= 0) & (i_ < H) & (j_ >= 0) & (j_ < W)
                z1 = ap * i_ + bb * j_ + e1 - di
                z2 = cc * i_ + dp * j_ + e2 - dj
                g += _tent(z1) * _tent(z2) * valid
        return g

    qs = np.arange(W, dtype=np.float64)
    ps = np.arange(1, H - 1, dtype=np.float64)
    ds = 0.0
    ds += np.sum(row0.astype(np.float64) * (g_true(0.0, qs) - kappa))
    ds += np.sum(row1.astype(np.float64) * (g_true(1023.0, qs) - kappa))
    ds += np.sum(c0[1:-1].astype(np.float64) * (g_true(ps, 0.0) - kappa))
    ds += np.sum(c1[1:-1].astype(np.float64) * (g_true(ps, 1023.0) - kappa))

    return kappa * float(sum_y) + ds


def _affine_f32(feat32, Wl, bl):
    M = (feat32 @ Wl + bl).reshape(3, 3)
    return np.eye(3, dtype=np.float32) + np.float32(0.01) * M


def kernel(x, y, Wpsi, bpsi, Wphi, bphi):
    from concourse import bass_utils

    B = x.shape[0]
    assert x.shape == (B, 1, H, W) and y.shape == (B, 1, H, W)

    nc = _get_program()
    in_maps = [
        {"x": np.ascontiguousarray(x[b, 0]), "y": np.ascontiguousarray(y[b, 0])}
        for b in range(B)
    ]
    results = bass_utils.run_bass_kernel_spmd(
        nc, in_maps, core_ids=list(range(B))
    ).results

    out = np.empty((B, 3, 3), dtype=np.float32)
    inv_hw = 1.0 / float(H * W)
    for b in range(B):
        r = np.asarray(results[b]["out"], dtype=np.float32).reshape(-1)
        sm = r[0:3584].reshape(128, SMALLS_COLS).astype(np.float64)
        sum_x = float(sm[:, 0:4].sum())
        sum_y = float(sm[:, 4:11].sum())
        # strip cols land block-major: sm[p, 12+blk] = y[blk*128 + p, 0]
        c0 = sm[:, 12:20].T.ravel()
        c1 = sm[:, 20:28].T.ravel()
        row0 = r[3584:4608].astype(np.float64)
        row1 = r[4608:5632].astype(np.float64)

        mean_x = np.float32(sum_x * inv_hw)
        mean_y = np.float32(sum_y * inv_hw)
        phi = _affine_f32(np.array([mean_x, mean_y], np.float32), Wpsi, bpsi)
        A = np.linalg.inv(phi)

        try:
            mean_yc = np.float32(_warp_sum(sum_y, row0, row1, c0, c1, A) * inv_hw)
        except AssertionError:
            mean_yc = np.float32(_warp_mean_exact(y[b, 0], A))

        psi = _affine_f32(np.array([mean_x, mean_yc], np.float32), Wphi, bphi)
        out[b] = phi + psi - np.eye(3, dtype=np.float32)
    return out


# revision 5
# speedup vs baseline: 1.0011x; 1.0011x over previous
"""Trainium2 kernel for nn_DoubleAffineNet.

Math: the module's output is phi + psi - I where phi, psi are 3x3 affine
matrices built from pooled image statistics. phi needs mean(x), mean(y).
psi needs mean(x) and mean(y_comp), where y_comp is y bilinearly warped by
the near-identity affine map phi^{-1}.

Key identity: only the MEAN of y_comp is needed. Writing the warp-mean as
sum_{p,q} Y[p,q] * G[p,q] (G = bilinear splat weights of the affinely
mapped output lattice), a partition-of-unity argument shows that for
sub-pixel displacement fields (|u|,|v| < 0.5, which holds for this
problem's near-identity maps; asserted at runtime on the host), G is the
constant kappa = (1-a')(1-d') + b*c everywhere except the four border
rows/cols. Hence

    sum(y_comp) = kappa * sum(y) + sum_border Y*(G_true - kappa)

The device kernel therefore only computes the memory-bound statistics:
per-sample sum(x), sum(y), and the four border strips of y. The remaining
O(B*(3x3 + 4*1024)) algebra runs on the host (f32 where the reference is
f32, f64 for the border correction).

Sharding: pure data parallel, one sample per NeuronCore (B=8, 8 cores).

Timing model (from NTFF traces): the measured exec window starts at the
framework's const-memset and ends after walrus's fixed epilogue (a
rendezvous + 253 per-semaphore clears split across the 5 sequencers +
a final ring, ~7us that cannot be overlapped or elided). The stream of
8 MB runs at ~390 GB/s on one HWDGE ring. What this kernel optimizes is
the chain between "last input byte lands" and "last engine reaches the
epilogue rendezvous":
  - the tail of y is split into two small [128,512] column chunks reduced
    concurrently by Scalar (ACT accum) and Vector (DVE reduce)
  - border-row strips go DRAM->DRAM on the otherwise-idle second HWDGE
    ring (Scalar's), issued up front
  - no engine waits for output-DMA receipts: the outputs land ~5us before
    the epilogue retires, so the receipt latency is hidden behind the
    semaphore sweep
"""

import numpy as np

H = 1024
W = 1024
OUT_LEN = 5376
SMALLS_COLS = 26

_CACHE = {}


def _build_program():
    import contextlib

    import concourse.bacc as bacc
    from concourse import mybir

    f32 = mybir.dt.float32
    Copy = mybir.ActivationFunctionType.Copy
    nc = bacc.Bacc("TRN2", target_bir_lowering=False, debug=False, num_devices=8)

    xd = nc.dram_tensor("x", [H, W], f32, kind="ExternalInput").ap()
    yd = nc.dram_tensor("y", [H, W], f32, kind="ExternalInput").ap()
    outd = nc.dram_tensor("out", [OUT_LEN], f32, kind="ExternalOutput").ap()

    # chunks (issue order = landing order, single Sync HWDGE ring):
    # c0..c3: x rows k*256..+255 as [128, 2, 1024] (1 MB each)
    # c4..c6: y rows 0..767 likewise (2 blocks of 128 rows each)
    # c7:     y rows 768..895 as [128, 1024] (0.5 MB)
    # c8:     y rows 896..1023, cols 0..511   [128, 512] (0.25 MB)
    # c9:     y rows 896..1023, cols 512..1023 [128, 512] (0.25 MB)
    def src(k):
        if k < 4:
            return xd[k * 256:(k + 1) * 256, :].rearrange("(a p) q -> p a q", a=2)
        if k < 7:
            c = k - 4
            return yd[c * 256:(c + 1) * 256, :].rearrange("(a p) q -> p a q", a=2)
        if k == 7:
            return yd[768:896, :]
        if k == 8:
            return yd[896:1024, 0:512]
        return yd[896:1024, 512:1024]

    free = [2048, 2048, 2048, 2048, 2048, 2048, 2048, 1024, 512, 512]

    # smalls cols: 0-3 x partials, 4-7 y partials (c4..c7),
    # 8-14 col0 strips blocks 0-6, 15-21 col1023 strips blocks 0-6,
    # 22 c8 partial, 23 c9 partial, 24 col0 block 7, 25 col1023 block 7
    with contextlib.ExitStack() as ctx:
        bufs = [
            ctx.enter_context(nc.sbuf_tensor(f"chunk{k}", [128, free[k]], f32))
            for k in range(10)
        ]
        smalls = ctx.enter_context(nc.sbuf_tensor("smalls", [128, SMALLS_COLS], f32))
        scratch = ctx.enter_context(nc.sbuf_tensor("scratch", [128, 2048], f32))
        sem_in = [ctx.enter_context(nc.semaphore(f"in{k}")) for k in range(10)]
        done = ctx.enter_context(nc.semaphore("done"))
        dma_out = ctx.enter_context(nc.semaphore("dma_out"))
        block = ctx.enter_context(nc.Block(no_gpsimd_drain=True))

        def dst(k):
            if k < 7:
                return bufs[k].ap().rearrange("p (a q) -> p a q", a=2)
            return bufs[k][:]

        @block.sync
        def _(sync):
            for k in range(10):
                sync.dma_start(out=dst(k), in_=src(k)).then_inc(sem_in[k], 16)
            # one smalls DMA once every reducer/strip has written its column;
            # nobody waits for its receipt — it lands ~5us before the NEFF's
            # fixed semaphore-sweep epilogue retires
            sync.wait_ge(done, 6)
            sync.dma_start(
                out=outd[0:3328].rearrange("(p c) -> p c", c=SMALLS_COLS),
                in_=smalls[:],
            ).then_inc(dma_out, 16)

        @block.scalar
        def _(scalar):
            # border-row strips, DRAM->DRAM on the otherwise-idle ACT ring
            scalar.dma_start(
                out=outd[3328:4352].rearrange("(p q) -> p q", p=1),
                in_=yd[0:1, :],
            ).then_inc(dma_out, 16)
            scalar.dma_start(
                out=outd[4352:5376].rearrange("(p q) -> p q", p=1),
                in_=yd[1023:1024, :],
            ).then_inc(dma_out, 16)
            # reduces via ACT accumulate
            for k in (0, 2, 4, 6):
                scalar.wait_ge(sem_in[k], 16)
                inst = nc.scalar.activation(
                    scratch[:, 0:free[k]], bufs[k][:], Copy,
                    accum_out=smalls[:, k:k + 1] if k < 4
                    else smalls[:, 4 + (k - 4):5 + (k - 4)],
                )
                if k == 6:
                    inst.then_inc(done, 1)
            scalar.wait_ge(sem_in[8], 16)
            nc.scalar.activation(
                scratch[:, 0:512], bufs[8][:], Copy,
                accum_out=smalls[:, 22:23],
            ).then_inc(done, 1)

        @block.vector
        def _(vector):
            for k in (1, 3, 5, 7):
                vector.wait_ge(sem_in[k], 16)
                col = k if k < 4 else 4 + (k - 4)
                inst = nc.vector.tensor_reduce(
                    out=smalls[:, col:col + 1],
                    in_=bufs[k][:],
                    axis=mybir.AxisListType.X,
                    op=mybir.AluOpType.add,
                )
                if k == 7:
                    inst.then_inc(done, 1)
            vector.wait_ge(sem_in[9], 16)
            nc.vector.tensor_reduce(
                out=smalls[:, 23:24],
                in_=bufs[9][:],
                axis=mybir.AxisListType.X,
                op=mybir.AluOpType.add,
            ).then_inc(done, 1)

        @block.gpsimd
        def _(gpsimd):
            # border-column strips; blocks 0-6 then block 7
            for k in (4, 5, 6):
                gpsimd.wait_ge(sem_in[k], 16)
                t3 = bufs[k].ap().rearrange("p (a q) -> p a q", a=2)
                j = 8 + 2 * (k - 4)
                nc.gpsimd.tensor_copy(smalls[:, j:j + 2], t3[:, :, 0])
                nc.gpsimd.tensor_copy(smalls[:, j + 7:j + 9], t3[:, :, 1023])
            gpsimd.wait_ge(sem_in[7], 16)
            nc.gpsimd.tensor_copy(smalls[:, 14:15], bufs[7][:, 0:1])
            nc.gpsimd.tensor_copy(
                smalls[:, 21:22], bufs[7][:, 1023:1024]
            ).then_inc(done, 1)
            gpsimd.wait_ge(sem_in[8], 16)
            nc.gpsimd.tensor_copy(smalls[:, 24:25], bufs[8][:, 0:1])
            gpsimd.wait_ge(sem_in[9], 16)
            nc.gpsimd.tensor_copy(
                smalls[:, 25:26], bufs[9][:, 511:512]
            ).then_inc(done, 1)

    nc.compile()
    return nc


def _get_program():
    if "nc" not in _CACHE:
        _CACHE["nc"] = _build_program()
    return _CACHE["nc"]


def _tent(z):
    return np.maximum(0.0, 1.0 - np.abs(z))


def _warp_mean_exact(y_img, A):
    """Fallback: honest bilinear warp-mean in numpy (used only if the
    sub-pixel displacement assumption fails, which it does not for this
    problem's inputs)."""
    A64 = A.astype(np.float64)
    i = np.arange(H, dtype=np.float64)[:, None]
    j = np.arange(W, dtype=np.float64)[None, :]
    px = A64[0, 0] * i + A64[0, 1] * j + 1023.0 * A64[0, 2]
    py = A64[1, 0] * i + A64[1, 1] * j + 1023.0 * A64[1, 2]
    x0 = np.floor(px).astype(np.int64)
    y0 = np.floor(py).astype(np.int64)
    wx = px - x0
    wy = py - y0
    im = y_img.astype(np.float64)
    acc = np.zeros((H, W))
    for xi, yi, w in (
        (x0, y0, (1 - wx) * (1 - wy)),
        (x0, y0 + 1, (1 - wx) * wy),
        (x0 + 1, y0, wx * (1 - wy)),
        (x0 + 1, y0 + 1, wx * wy),
    ):
        valid = (xi >= 0) & (xi < H) & (yi >= 0) & (yi < W)
        acc += im[np.clip(xi, 0, H - 1), np.clip(yi, 0, W - 1)] * w * valid
    return acc.mean()


def _warp_sum(sum_y, row0, row1, c0, c1, A):
    """sum(y_comp) from sum(y) + border strips, given phi_inv = A (f32).

    Requires the sub-pixel displacement assumption |u|,|v| < 0.5 (checked
    at the field corners; the fields are affine so corners bound the
    interior). The caller falls back to _warp_mean_exact otherwise.
    """
    A64 = A.astype(np.float64)
    ap, bb = A64[0, 0] - 1.0, A64[0, 1]
    cc, dp = A64[1, 0], A64[1, 1] - 1.0
    e1, e2 = 1023.0 * A64[0, 2], 1023.0 * A64[1, 2]

    mu = max(abs(ap * i + bb * j + e1) for i in (0.0, 1023.0) for j in (0.0, 1023.0))
    mv = max(abs(cc * i + dp * j + e2) for i in (0.0, 1023.0) for j in (0.0, 1023.0))
    assert mu < 0.5 and mv < 0.5, (mu, mv)

    kappa = (1.0 - ap) * (1.0 - dp) + bb * cc

    def g_true(p, q):
        g = np.zeros(np.broadcast(p, q).shape)
        for di in (-1, 0, 1):
            for dj in (-1, 0, 1):
                i_, j_ = p - di, q - dj
                valid = (i_ >= 0) & (i_ < H) & (j_ >= 0) & (j_ < W)
                z1 = ap * i_ + bb * j_ + e1 - di
                z2 = cc * i_ + dp * j_ + e2 - dj
                g += _tent(z1) * _tent(z2) * valid
        return g

    qs = np.arange(W, dtype=np.float64)
    ps = np.arange(1, H - 1, dtype=np.float64)
    ds = 0.0
    ds += np.sum(row0.astype(np.float64) * (g_true(0.0, qs) - kappa))
    ds += np.sum(row1.astype(np.float64) * (g_true(1023.0, qs) - kappa))
    ds += np.sum(c0[1:-1].astype(np.float64) * (g_true(ps, 0.0) - kappa))
    ds += np.sum(c1[1:-1].astype(np.float64) * (g_true(ps, 1023.0) - kappa))

    return kappa * float(sum_y) + ds


def _affine_f32(feat32, Wl, bl):
    M = (feat32 @ Wl + bl).reshape(3, 3)
    return np.eye(3, dtype=np.float32) + np.float32(0.01) * M


def kernel(x, y, Wpsi, bpsi, Wphi, bphi):
    from concourse import bass_utils

    B = x.shape[0]
    assert x.shape == (B, 1, H, W) and y.shape == (B, 1, H, W)

    nc = _get_program()
    in_maps = [
        {"x": np.ascontiguousarray(x[b, 0]), "y": np.ascontiguousarray(y[b, 0])}
        for b in range(B)
    ]
    results = bass_utils.run_bass_kernel_spmd(
        nc, in_maps, core_ids=list(range(B))
    ).results

    out = np.empty((B, 3, 3), dtype=np.float32)
    inv_hw = 1.0 / float(H * W)
    for b in range(B):
        r = np.asarray(results[b]["out"], dtype=np.float32).reshape(-1)
        sm = r[0:3328].reshape(128, SMALLS_COLS).astype(np.float64)
        sum_x = float(sm[:, 0:4].sum())
        sum_y = float(sm[:, 4:8].sum() + sm[:, 22:24].sum())
        # strip cols: sm[p, 8+blk] = y[blk*128+p, 0] for blk 0-6; block 7 in
        # col 24; col-1023 strips likewise at 15+blk and 25
        c0 = np.concatenate([sm[:, 8:15].T.ravel(), sm[:, 24]])
        c1 = np.concatenate([sm[:, 15:22].T.ravel(), sm[:, 25]])
        row0 = r[3328:4352].astype(np.float64)
        row1 = r[4352:5376].astype(np.float64)

        mean_x = np.float32(sum_x * inv_hw)
        mean_y = np.float32(sum_y * inv_hw)
        phi = _affine_f32(np.array([mean_x, mean_y], np.float32), Wpsi, bpsi)
        A = np.linalg.inv(phi)

        try:
            mean_yc = np.float32(_warp_sum(sum_y, row0, row1, c0, c1, A) * inv_hw)
        except AssertionError:
            mean_yc = np.float32(_warp_mean_exact(y[b, 0], A))

        psi = _affine_f32(np.array([mean_x, mean_yc], np.float32), Wphi, bphi)
        out[b] = phi + psi - np.eye(3, dtype=np.float32)
    return out


# revision 9
# speedup vs baseline: 1.0611x; 1.0599x over previous
"""Trainium2 kernel for nn_DoubleAffineNet.

Math: the module's output is phi + psi - I where phi, psi are 3x3 affine
matrices built from pooled image statistics. phi needs mean(x), mean(y).
psi needs mean(x) and mean(y_comp), where y_comp is y bilinearly warped by
the near-identity affine map phi^{-1}.

Key identity: only the MEAN of y_comp is needed. Writing the warp-mean as
sum_{p,q} Y[p,q] * G[p,q] (G = bilinear splat weights of the affinely
mapped output lattice), a partition-of-unity argument shows that for
sub-pixel displacement fields (|u|,|v| < 0.5, which holds for this
problem's near-identity maps; asserted at runtime on the host), G is the
constant kappa = (1-a')(1-d') + b*c everywhere except the four border
rows/cols. Hence

    sum(y_comp) = kappa * sum(y) + sum_border Y*(G_true - kappa)

The device kernel therefore only computes the memory-bound statistics:
per-sample sum(x), sum(y), and the four border strips of y. The remaining
O(B*(3x3 + 4*1024)) algebra runs on the host (f32 where the reference is
f32, f64 for the border correction).

Sharding: pure data parallel, one sample per NeuronCore (B=8, 8 cores).

Timing model (from NTFF traces): the measured exec window starts at the
framework's const-memsets and ends after the runtime's fixed epilogue
(rendezvous + 253 per-semaphore clears split across the 5 sequencers +
final ring, ~7us — injected by NRT at NEFF load, cannot be changed).
The 8 MB input stream runs at ~390 GB/s on the single Sync HWDGE ring.
This kernel minimizes the serial chain between "last input byte lands"
and "all engines reach the epilogue rendezvous":
  - y streams FIRST so its border strips (row/col) are extracted
    mid-stream from SBUF, completely off the critical path
  - x streams LAST, its tail split into two [128,512] column chunks
    reduced concurrently by Scalar (ACT accum) and Vector (DVE)
  - no engine waits for the output DMA's receipt: the smalls land ~5us
    before the epilogue retires, hiding the HBM write latency
  - everything stays on one HWDGE ring (a second ring measurably
    disrupts the SDMA packet round-robin and slows the stream)
"""

import numpy as np

H = 1024
W = 1024
OUT_LEN = 5632
SMALLS_COLS = 28

_CACHE = {}


def _build_program():
    import contextlib

    import concourse.bacc as bacc
    from concourse import mybir

    f32 = mybir.dt.float32
    Copy = mybir.ActivationFunctionType.Copy
    nc = bacc.Bacc("TRN2", target_bir_lowering=False, debug=False, num_devices=8)

    xd = nc.dram_tensor("x", [H, W], f32, kind="ExternalInput").ap()
    yd = nc.dram_tensor("y", [H, W], f32, kind="ExternalInput").ap()
    outd = nc.dram_tensor("out", [OUT_LEN], f32, kind="ExternalOutput").ap()

    # chunks (issue order = landing order, single Sync HWDGE ring):
    # c0..c2: y rows k*256..+255 as [128, 2, 1024] (1 MB each, blocks 2k,2k+1)
    # c3:     y rows 768..895  [128, 1024] (block 6)
    # c4:     y rows 896..1023 [128, 1024] (block 7; its partition 127 is
    #         row 1023, DMA'd out directly as the border-row strip)
    # c5..c7: x rows k*256..+255 as [128, 2, 1024]
    # c8:     x rows 768..895  [128, 1024]
    # c9:     x rows 896..1023, cols 0..511   [128, 512]
    # c10:    x rows 896..1023, cols 512..1023 [128, 512]
    def src(k):
        if k < 3:
            return yd[k * 256:(k + 1) * 256, :].rearrange("(a p) q -> p a q", a=2)
        if k == 3:
            return yd[768:896, :]
        if k == 4:
            return yd[896:1024, :]
        if k < 8:
            c = k - 5
            return xd[c * 256:(c + 1) * 256, :].rearrange("(a p) q -> p a q", a=2)
        if k == 8:
            return xd[768:896, :]
        if k == 9:
            return xd[896:1024, 0:768]
        return xd[896:1024, 768:1024]

    free = [2048, 2048, 2048, 1024, 1024, 2048, 2048, 2048, 1024, 768, 256]

    # smalls cols: 0-4 y partials (c0..c4), 5-10 x partials (c5..c10),
    # 11-18 col0 strips (blocks 0-7), 19-26 col1023 strips, 27 unused
    with contextlib.ExitStack() as ctx:
        bufs = [
            ctx.enter_context(nc.sbuf_tensor(f"chunk{k}", [128, free[k]], f32))
            for k in range(11)
        ]
        smalls = ctx.enter_context(nc.sbuf_tensor("smalls", [128, SMALLS_COLS], f32))
        scratch = ctx.enter_context(nc.sbuf_tensor("scratch", [128, 2048], f32))
        sem_in = [ctx.enter_context(nc.semaphore(f"in{k}")) for k in range(11)]
        done = ctx.enter_context(nc.semaphore("done"))
        dma_out = ctx.enter_context(nc.semaphore("dma_out"))
        block = ctx.enter_context(nc.Block(no_gpsimd_drain=True))

        def dst(k):
            if free[k] == 2048:
                return bufs[k].ap().rearrange("p (a q) -> p a q", a=2)
            return bufs[k][:]

        @block.sync
        def _(sync):
            for k in range(11):
                sync.dma_start(out=dst(k), in_=src(k)).then_inc(sem_in[k], 16)
            # border-row strips out of the resident y chunks, mid-stream
            sync.wait_ge(sem_in[0], 16)
            sync.dma_start(
                out=outd[3584:4608].rearrange("(p q) -> p q", p=1),
                in_=bufs[0][0:1, 0:W],
            ).then_inc(dma_out, 16)
            sync.wait_ge(sem_in[4], 16)
            sync.dma_start(
                out=outd[4608:5632].rearrange("(p q) -> p q", p=1),
                in_=bufs[4][127:128, :],
            ).then_inc(dma_out, 16)
            # one smalls DMA once every reducer/strip wrote its column;
            # nobody waits for receipts — the data lands ~5us before the
            # runtime's fixed semaphore-sweep epilogue retires
            sync.wait_ge(done, 3)
            sync.dma_start(
                out=outd[0:3584].rearrange("(p c) -> p c", c=SMALLS_COLS),
                in_=smalls[:],
            ).then_inc(dma_out, 16)

        @block.scalar
        def _(scalar):
            # ACT-accumulate reduces; c10 is the smallest, last-landing chunk
            sc = (0, 2, 4, 6, 8, 10)
            for j, k in enumerate(sc):
                scalar.wait_ge(sem_in[k], 16)
                inst = nc.scalar.activation(
                    scratch[:, 0:free[k]], bufs[k][:], Copy,
                    accum_out=smalls[:, k:k + 1],
                )
                if j == len(sc) - 1:
                    inst.then_inc(done, 1)

        @block.vector
        def _(vector):
            vc = (1, 3, 5, 7, 9)
            for j, k in enumerate(vc):
                vector.wait_ge(sem_in[k], 16)
                inst = nc.vector.tensor_reduce(
                    out=smalls[:, k:k + 1],
                    in_=bufs[k][:],
                    axis=mybir.AxisListType.X,
                    op=mybir.AluOpType.add,
                )
                if j == len(vc) - 1:
                    inst.then_inc(done, 1)

        @block.gpsimd
        def _(gpsimd):
            # border-column strips from the resident y chunks
            for k in (0, 1, 2):
                gpsimd.wait_ge(sem_in[k], 16)
                t3 = bufs[k].ap().rearrange("p (a q) -> p a q", a=2)
                j = 11 + 2 * k
                nc.gpsimd.tensor_copy(smalls[:, j:j + 2], t3[:, :, 0])
                nc.gpsimd.tensor_copy(smalls[:, j + 8:j + 10], t3[:, :, 1023])
            gpsimd.wait_ge(sem_in[3], 16)
            nc.gpsimd.tensor_copy(smalls[:, 17:18], bufs[3][:, 0:1])
            nc.gpsimd.tensor_copy(smalls[:, 25:26], bufs[3][:, 1023:1024])
            gpsimd.wait_ge(sem_in[4], 16)
            nc.gpsimd.tensor_copy(smalls[:, 18:19], bufs[4][:, 0:1])
            nc.gpsimd.tensor_copy(
                smalls[:, 26:27], bufs[4][:, 1023:1024]
            ).then_inc(done, 1)

    nc.compile()
    return nc


def _get_program():
    if "nc" not in _CACHE:
        _CACHE["nc"] = _build_program()
    return _CACHE["nc"]


def _tent(z):
    return np.maximum(0.0, 1.0 - np.abs(z))


def _warp_mean_exact(y_img, A):
    """Fallback: honest bilinear warp-mean in numpy (used only if the
    sub-pixel displacement assumption fails, which it does not for this
    problem's inputs)."""
    A64 = A.astype(np.float64)
    i = np.arange(H, dtype=np.float64)[:, None]
    j = np.arange(W, dtype=np.float64)[None, :]
    px = A64[0, 0] * i + A64[0, 1] * j + 1023.0 * A64[0, 2]
    py = A64[1, 0] * i + A64[1, 1] * j + 1023.0 * A64[1, 2]
    x0 = np.floor(px).astype(np.int64)
    y0 = np.floor(py).astype(np.int64)
    wx = px - x0
    wy = py - y0
    im = y_img.astype(np.float64)
    acc = np.zeros((H, W))
    for xi, yi, w in (
        (x0, y0, (1 - wx) * (1 - wy)),
        (x0, y0 + 1, (1 - wx) * wy),
        (x0 + 1, y0, wx * (1 - wy)),
        (x0 + 1, y0 + 1, wx * wy),
    ):
        valid = (xi >= 0) & (xi < H) & (yi >= 0) & (yi < W)
        acc += im[np.clip(xi, 0, H - 1), np.clip(yi, 0, W - 1)] * w * valid
    return acc.mean()


def _warp_sum(sum_y, row0, row1, c0, c1, A):
    """sum(y_comp) from sum(y) + border strips, given phi_inv = A (f32).

    Requires the sub-pixel displacement assumption |u|,|v| < 0.5 (checked
    at the field corners; the fields are affine so corners bound the
    interior). The caller falls back to _warp_mean_exact otherwise.
    """
    A64 = A.astype(np.float64)
    ap, bb = A64[0, 0] - 1.0, A64[0, 1]
    cc, dp = A64[1, 0], A64[1, 1] - 1.0
    e1, e2 = 1023.0 * A64[0, 2], 1023.0 * A64[1, 2]

    mu = max(abs(ap * i + bb * j + e1) for i in (0.0, 1023.0) for j in (0.0, 1023.0))
    mv = max(abs(cc * i + dp * j + e2) for i in (0.0, 1023.0) for j in (0.0, 1023.0))
    assert mu < 0.5 and mv < 0.5, (mu, mv)

    kappa = (1.0 - ap) * (1.0 - dp) + bb * cc

    def g_true(p, q):
        g = np.zeros(np.broadcast(p, q).shape)
        for di in (-1, 0, 1):
            for dj in (-1, 0, 1):
                i_, j_ = p - di, q - dj
                valid = (i_ >= 0) & (i_ < H) & (j_ >= 0) & (j_ < W)
                z1 = ap * i_ + bb * j_ + e1 - di
                z2 = cc * i_ + dp * j_ + e2 - dj
                g += _tent(z1) * _tent(z2) * valid
        return g

    qs = np.arange(W, dtype=np.float64)
    ps = np.arange(1, H - 1, dtype=np.float64)
    ds = 0.0
    ds += np.sum(row0.astype(np.float64) * (g_true(0.0, qs) - kappa))
    ds += np.sum(row1.astype(np.float64) * (g_true(1023.0, qs) - kappa))
    ds += np.sum(c0[1:-1].astype(np.float64) * (g_true(ps, 0.0) - kappa))
    ds += np.sum(c1[1:-1].astype(np.float64) * (g_true(ps, 1023.0) - kappa))

    return kappa * float(sum_y) + ds


def _affine_f32(feat32, Wl, bl):
    M = (feat32 @ Wl + bl).reshape(3, 3)
    return np.eye(3, dtype=np.float32) + np.float32(0.01) * M


def kernel(x, y, Wpsi, bpsi, Wphi, bphi):
    from concourse import bass_utils

    B = x.shape[0]
    assert x.shape == (B, 1, H, W) and y.shape == (B, 1, H, W)

    nc = _get_program()
    in_maps = [
        {"x": np.ascontiguousarray(x[b, 0]), "y": np.ascontiguousarray(y[b, 0])}
        for b in range(B)
    ]
    results = bass_utils.run_bass_kernel_spmd(
        nc, in_maps, core_ids=list(range(B))
    ).results

    out = np.empty((B, 3, 3), dtype=np.float32)
    inv_hw = 1.0 / float(H * W)
    for b in range(B):
        r = np.asarray(results[b]["out"], dtype=np.float32).reshape(-1)
        sm = r[0:3584].reshape(128, SMALLS_COLS).astype(np.float64)
        sum_y = float(sm[:, 0:5].sum())
        sum_x = float(sm[:, 5:11].sum())
        # strip cols land block-major: sm[p, 11+blk] = y[blk*128+p, 0]
        c0 = sm[:, 11:19].T.ravel()
        c1 = sm[:, 19:27].T.ravel()
        row0 = r[3584:4608].astype(np.float64)
        row1 = r[4608:5632].astype(np.float64)

        mean_x = np.float32(sum_x * inv_hw)
        mean_y = np.float32(sum_y * inv_hw)
        phi = _affine_f32(np.array([mean_x, mean_y], np.float32), Wpsi, bpsi)
        A = np.linalg.inv(phi)

        try:
            mean_yc = np.float32(_warp_sum(sum_y, row0, row1, c0, c1, A) * inv_hw)
        except AssertionError:
            mean_yc = np.float32(_warp_mean_exact(y[b, 0], A))

        psi = _affine_f32(np.array([mean_x, mean_yc], np.float32), Wphi, bphi)
        out[b] = phi + psi - np.eye(3, dtype=np.float32)
    return out
